# revision 1
# baseline (speedup 1.0000x reference)
"""Trainium2 Bass kernel for DeepMultiOmicPathwayNet.

Model (per batch row n):
  g    = x[n, pathway_ids, :]                  -> [P=200, K*C=192]
  t    = einsum('pi,pio->po', g, W_path) + b_path      (per-pathway linear)
  t    = t / ||t||_2 (row L2 over each pathway's 64 outputs)
  ncb  = x[n, nc_ids, :].flatten() @ W_nc + b_nc       ([15000] @ [15000,512])
  h    = sigmoid(concat(t.flatten(), ncb))             ([13312])
  out  = h @ W_out + b_out                             ([20])

Strategy: data-parallel over batch N=1024 across 8 cores (128 rows/core).
Host prep does the (compile-time-known) gathers + transposes + bf16 cast and
folds every bias into the matmuls by appending a ones-row to the data and the
bias as an extra contraction row of the weights.

Perf notes (from NTFF profiles):
  - DMA issue is ~0.7us per dma_start serialized on the SP queue, but one DMA's
    per-partition packets spread across all 16 DMA engines. So: few, large DMAs
    (2-8KB per partition line), grouped 4 pathway-pairs / 8 nc K-tiles each.
  - PE is_transpose costs ~580ns per [128,128]; a regular matmul against a
    bf16 identity (s.T = lhsT.T @ I) costs ~220ns -> used for all transposes.
  - ACT has ~300ns/instr overhead: sigmoid batched per 4 pairs [128,512];
    sqrt batched over all 200 pathway norms in one op (also avoids
    per-pathway activation-table thrash).
  - Per-pathway sum-of-squares: chunked DVE square (tensor_mul) + tensor_reduce
    over 8 pathways at a time (tensor_tensor_reduce crashes the exec unit;
    per-pathway ACT Square+accum costs 2 instrs/pathway).
  - PSUM->SBUF bf16 casts alternate DVE/ACT to balance engine load.
"""
import numpy as np
import ml_dtypes

import concourse.bass as bass
import concourse.bacc as bacc
import concourse.tile as tile
import concourse.mybir as mybir
from concourse.bass_utils import run_bass_kernel_spmd
from concourse.masks import make_identity

bf16 = mybir.dt.bfloat16
f32 = mybir.dt.float32
BF = ml_dtypes.bfloat16
AF = mybir.ActivationFunctionType

N, G, C = 1024, 20000, 3
P, K = 200, 64
KC = K * C              # 192
NCG = 5000              # non-cancer genes
HID = 512
OUT = 20
NB = 128                # batch rows per core
NCORES = 8
NPAIR = P // 2          # 100 pathway pairs
NGRP = NPAIR // 4       # 25 groups of 4 pairs (DMA granularity)
NCHUNK = 15             # nc-branch K chunks
NKT = NCHUNK * 8        # 120 K tiles of 128: 15360 >= 15001
NKROWS = NKT * 128
NFT = P * K // 128      # 100 feature tiles from pathways
NFT_NC = HID // 128     # 4 feature tiles from nc branch
FEAT = P * K + HID      # 13312

_CACHE = {}


def _build(npair=NPAIR, nchunk=NCHUNK, nft_nc=NFT_NC):
    nc = bacc.Bacc(None, target_bir_lowering=False)

    pd_hi_d = nc.declare_dram_parameter("pd_hi", [NGRP, 128, 1024], bf16, isOutput=False)
    pd_lo_d = nc.declare_dram_parameter("pd_lo", [NGRP, 65, 1024], bf16, isOutput=False)
    wphi_d = nc.declare_dram_parameter("wphi", [128, P, K], bf16, isOutput=False)
    wplo_d = nc.declare_dram_parameter("wplo", [65, P, K], bf16, isOutput=False)
    ncd_d = nc.declare_dram_parameter("ncd", [NCHUNK, 128, 1024], bf16, isOutput=False)
    wnc_d = nc.declare_dram_parameter("wnc", [NCHUNK, 128, 4096], bf16, isOutput=False)
    wout_d = nc.declare_dram_parameter("wout", [128, NFT + NFT_NC, OUT], bf16, isOutput=False)
    bout_d = nc.declare_dram_parameter("bout", [1, OUT], bf16, isOutput=False)
    out_d = nc.declare_dram_parameter("out", [NB, OUT], f32, isOutput=True)

    with tile.TileContext(nc) as tc:
        with (
            tc.tile_pool(name="cst", bufs=1) as cst,
            tc.tile_pool(name="pd", bufs=2) as pd,
            tc.tile_pool(name="ncw", bufs=2) as ncw,
            tc.tile_pool(name="sig", bufs=2) as sig,
            tc.tile_pool(name="tp", bufs=2, space="PSUM") as tp,
            tc.tile_pool(name="stp", bufs=2, space="PSUM") as stp,
            tc.tile_pool(name="ncp", bufs=1, space="PSUM") as ncp,
            tc.tile_pool(name="outp", bufs=1, space="PSUM") as outp,
        ):
            ident = cst.tile([128, 128], bf16)
            make_identity(nc, ident[:])
            ones_t = cst.tile([1, 128], bf16)
            nc.gpsimd.memset(ones_t[:], 1.0)

            wphi_sb = cst.tile([128, P, K], bf16)
            nc.sync.dma_start(wphi_sb[:], wphi_d[:])
            wplo_sb = cst.tile([65, P, K], bf16)
            nc.sync.dma_start(wplo_sb[:], wplo_d[:])
            wout_sb = cst.tile([128, NFT + NFT_NC, OUT], bf16)
            nc.sync.dma_start(wout_sb[:], wout_d[:])
            bout_sb = cst.tile([1, OUT], bf16)
            nc.sync.dma_start(bout_sb[:], bout_d[:])

            t_all = cst.tile([NB, P, K], bf16)
            ss_all = cst.tile([NB, P], f32)
            inv_all = cst.tile([NB, P], f32)

            # ---------- phase A: pathway matmuls + chunked sum-of-squares ----------
            pdh4 = pdl4 = None
            for j in range(npair):
                g, q = divmod(j, 4)
                if q == 0:
                    pdh4 = pd.tile([128, 4, 2, 128], bf16)
                    nc.sync.dma_start(pdh4[:], pd_hi_d[g])
                    pdl4 = pd.tile([65, 4, 2, 128], bf16)
                    nc.sync.dma_start(pdl4[:], pd_lo_d[g])
                t_ps = tp.tile([NB, 2, K], f32)
                for jj in range(2):
                    p = 2 * j + jj
                    nc.tensor.matmul(t_ps[:, jj, :], pdh4[:, q, jj, :],
                                     wphi_sb[:, p, :], start=True, stop=False)
                    nc.tensor.matmul(t_ps[:, jj, :], pdl4[:, q, jj, :],
                                     wplo_sb[:, p, :], start=False, stop=True)
                nc.vector.tensor_copy(t_all[:, 2 * j:2 * j + 2, :], t_ps[:])
                if q == 3:  # pathways 8g .. 8g+7 now in t_all
                    sq8 = pd.tile([NB, 8, K], bf16)
                    nc.vector.tensor_mul(sq8[:], t_all[:, 8 * g:8 * g + 8, :],
                                         t_all[:, 8 * g:8 * g + 8, :])
                    nc.vector.tensor_reduce(ss_all[:, 8 * g:8 * g + 8], sq8[:],
                                            axis=mybir.AxisListType.X,
                                            op=mybir.AluOpType.add)

            # ---------- phase B: 1/norm, batched (single sqrt table load) ----------
            nc.scalar.sqrt(inv_all[:], ss_all[:])
            nc.vector.reciprocal(inv_all[:], inv_all[:])

            # ---------- phase C: nc-branch matmuls interleaved with pathway finalize --
            nc_ps = ncp.tile([NB, HID], f32)
            out_ps = outp.tile([NB, OUT], f32)
            nkt = nchunk * 8
            ncd_t = wnc_t = s4 = None
            for step in range(nkt):
                ck, i = divmod(step, 8)
                if i == 0:
                    ncd_t = ncw.tile([128, 8, NB], bf16)
                    nc.sync.dma_start(ncd_t[:], ncd_d[ck])
                    wnc_t = ncw.tile([128, 8, HID], bf16)
                    nc.sync.dma_start(wnc_t[:], wnc_d[ck])

                if step < npair:
                    j = step
                    g2, q2 = divmod(j, 4)
                    if q2 == 0:
                        tn4 = sig.tile([NB, 8, K], bf16)
                        nc.vector.tensor_mul(
                            tn4[:], t_all[:, 8 * g2:8 * g2 + 8, :],
                            inv_all[:, 8 * g2:8 * g2 + 8].broadcast_to((NB, 8, K)))
                        s4 = sig.tile([NB, 8, K], bf16)
                        nc.scalar.activation(s4[:], tn4[:], AF.Sigmoid)
                    st_ps = stp.tile([128, NB], f32)
                    nc.tensor.matmul(st_ps[:], s4[:, 2 * q2:2 * q2 + 2, :], ident[:],
                                     start=True, stop=True)
                    hT = sig.tile([128, NB], bf16)
                    if j % 2 == 0:
                        nc.vector.tensor_copy(hT[:], st_ps[:])
                    else:
                        nc.scalar.copy(hT[:], st_ps[:])
                    nc.tensor.matmul(out_ps[:], hT[:], wout_sb[:, j, :],
                                     start=(j == 0), stop=False)

                nc.tensor.matmul(nc_ps[:], ncd_t[:, i, :], wnc_t[:, i, :],
                                 start=(step == 0), stop=(step == nkt - 1))

            # ---------- tail: nc sigmoid, transposes, final tiles, bias, out --------
            s_nc = cst.tile([NB, HID], bf16)
            nc.scalar.activation(s_nc[:], nc_ps[:], AF.Sigmoid)
            for i in range(nft_nc):
                st_ps = stp.tile([128, NB], f32)
                nc.tensor.matmul(st_ps[:], s_nc[:, i * 128:(i + 1) * 128], ident[:],
                                 start=True, stop=True)
                hT = sig.tile([128, NB], bf16)
                if i % 2 == 0:
                    nc.vector.tensor_copy(hT[:], st_ps[:])
                else:
                    nc.scalar.copy(hT[:], st_ps[:])
                nc.tensor.matmul(out_ps[:], hT[:], wout_sb[:, NFT + i, :],
                                 start=False, stop=False)
            nc.tensor.matmul(out_ps[:], ones_t[:], bout_sb[:],
                             start=False, stop=True)

            out_sb = cst.tile([NB, OUT], f32)
            nc.vector.tensor_copy(out_sb[:], out_ps[:])
            nc.sync.dma_start(out_d[:], out_sb[:])

    nc.compile()
    return nc


def _prep(inputs):
    x = np.asarray(inputs["x"], np.float32)
    pathway_ids = np.asarray(inputs["pathway_ids"]).astype(np.int64)
    nc_ids = np.asarray(inputs["nc_ids"]).astype(np.int64)
    W_path = np.asarray(inputs["W_path"], np.float32)
    b_path = np.asarray(inputs["b_path"], np.float32)
    W_nc = np.asarray(inputs["W_nc"], np.float32)
    b_nc = np.asarray(inputs["b_nc"], np.float32)
    W_out = np.asarray(inputs["W_out"], np.float32)
    b_out = np.asarray(inputs["b_out"], np.float32)

    n = x.shape[0]
    xt = np.ascontiguousarray(x.reshape(n, G * C).T)            # [60000, n]

    pidx = ((pathway_ids * 3)[:, :, None] + np.arange(3)).reshape(-1)
    prows = xt[pidx].reshape(P, KC, n)                          # [200, 192, n]
    ph = prows[:, 0:128, :]                                     # [200, 128, n]
    pl = np.concatenate([prows[:, 128:KC, :], np.ones((P, 1, n), np.float32)], axis=1)
    # [NGRP, rows, 4(pair-in-grp), 2(path-in-pair), n]
    ph_g = np.ascontiguousarray(ph.reshape(NGRP, 4, 2, 128, n).transpose(0, 3, 1, 2, 4)).astype(BF)
    pl_g = np.ascontiguousarray(pl.reshape(NGRP, 4, 2, 65, n).transpose(0, 3, 1, 2, 4)).astype(BF)

    nidx = ((nc_ids * 3)[:, None] + np.arange(3)).reshape(-1)
    ncd_all = np.zeros((NKROWS, n), np.float32)
    ncd_all[:NCG * C] = xt[nidx]
    ncd_all[NCG * C] = 1.0
    # [NCHUNK, 128, 8, n]
    ncd_all = np.ascontiguousarray(
        ncd_all.reshape(NCHUNK, 8, 128, n).transpose(0, 2, 1, 3)).astype(BF)

    wphi = np.ascontiguousarray(W_path[:, 0:128, :].transpose(1, 0, 2)).astype(BF)
    wplo = np.ascontiguousarray(
        np.concatenate([W_path[:, 128:KC, :], b_path[:, None, :]], axis=1).transpose(1, 0, 2)
    ).astype(BF)
    wnc_aug = np.zeros((NKROWS, HID), np.float32)
    wnc_aug[:NCG * C] = W_nc
    wnc_aug[NCG * C] = b_nc
    wnc_aug = np.ascontiguousarray(
        wnc_aug.reshape(NCHUNK, 8, 128, HID).transpose(0, 2, 1, 3)).astype(BF)  # [15,128,8,512]
    wout_t = np.ascontiguousarray(
        W_out.reshape(NFT + NFT_NC, 128, OUT).transpose(1, 0, 2)
    ).astype(BF)
    bout = b_out.reshape(1, OUT).astype(BF)

    in_maps = []
    for c in range(NCORES):
        sl = slice(c * NB, (c + 1) * NB)
        in_maps.append({
            "pd_hi": np.ascontiguousarray(ph_g[:, :, :, :, sl]).reshape(NGRP, 128, 1024),
            "pd_lo": np.ascontiguousarray(pl_g[:, :, :, :, sl]).reshape(NGRP, 65, 1024),
            "wphi": wphi,
            "wplo": wplo,
            "ncd": np.ascontiguousarray(ncd_all[:, :, :, sl]).reshape(NCHUNK, 128, 1024),
            "wnc": wnc_aug.reshape(NCHUNK, 128, 4096),
            "wout": wout_t,
            "bout": bout,
        })
    return in_maps


def kernel(**inputs):
    if "nc" not in _CACHE:
        _CACHE["nc"] = _build()
    nc = _CACHE["nc"]
    in_maps = _prep(inputs)
    res = run_bass_kernel_spmd(nc, in_maps, list(range(NCORES)), **_CACHE.get("run_kwargs", {}))
    _CACHE["last_result"] = res
    return np.concatenate([res.results[c]["out"] for c in range(NCORES)], axis=0)


if __name__ == "__main__":
    print("building only...")
    _build()
    print("build OK")



# revision 2
# speedup vs baseline: 1.7597x; 1.7597x over previous
"""Trainium2 Bass kernel for DeepMultiOmicPathwayNet.

Model (per batch row n):
  g    = x[n, pathway_ids, :]                  -> [P=200, K*C=192]
  t    = einsum('pi,pio->po', g, W_path) + b_path      (per-pathway linear)
  t    = t / ||t||_2 (row L2 over each pathway's 64 outputs)
  ncb  = x[n, nc_ids, :].flatten() @ W_nc + b_nc       ([15000] @ [15000,512])
  h    = sigmoid(concat(t.flatten(), ncb))             ([13312])
  out  = h @ W_out + b_out                             ([20])

Strategy: data-parallel over batch N=1024 across 8 cores (128 rows/core).
This is a DMA-bound problem; the v2 design cuts HBM bytes ~2x via fp8:
  - All large tensors (gathered x rows, W_path, W_nc) ship as fp8e4m3.
    W_path/b_path are pre-scaled x8 (exactly cancelled by the L2 norm);
    W_nc/b_nc are pre-scaled x64 (compensated via sigmoid's scale=1/64).
    W_out stays bf16 (too small-valued for fp8 subnormals, and tiny anyway).
  - nc-branch matmuls use fp8 DoubleRow perf mode (2 k-tiles per matmul at
    0.5 cycles/row -> 4x fewer PE cycles than bf16).
  - ~25 large up-front DMAs (SP-issue ~0.6us each is off the critical path)
    instead of ~90; weights/data land in persistent SBUF tiles.
  - Elementwise pipeline split across ACT (square, sigmoid) and DVE
    (reduce, normalize-mul) with PSUM->SBUF copies alternating engines.
  - One batched sqrt (ACT table swap x2 total) between phase A and B.
"""
import numpy as np
import ml_dtypes

import concourse.bass as bass
import concourse.bacc as bacc
import concourse.tile as tile
import concourse.mybir as mybir
from concourse.bass_utils import run_bass_kernel_spmd
from concourse.masks import make_identity

bf16 = mybir.dt.bfloat16
f32 = mybir.dt.float32
f8 = mybir.dt.float8e4
BF = ml_dtypes.bfloat16
F8 = ml_dtypes.float8_e4m3
AF = mybir.ActivationFunctionType
DR = mybir.MatmulPerfMode.DoubleRow

N, G, C = 1024, 20000, 3
P, K = 200, 64
KC = K * C              # 192
NCG = 5000              # non-cancer genes
HID = 512
OUT = 20
NB = 128                # batch rows per core
NCORES = 8
NGRP = P // 8           # 25 groups of 8 pathways
NKT = 118               # nc-branch k-tiles of 128 rows: 15104 >= 15001
NKP = NKT // 2          # 59 DoubleRow k-tile pairs
NKROWS = NKT * 128
NFT = P * K // 128      # 100 feature tiles from pathways
NFT_NC = HID // 128     # 4 feature tiles from nc branch

_CACHE = {}


def _build():
    nc = bacc.Bacc(None, target_bir_lowering=False)

    pdhi_d = nc.declare_dram_parameter("pdhi", [128, NGRP, 8, 128], f8, isOutput=False)
    pdlo_d = nc.declare_dram_parameter("pdlo", [65, NGRP, 8, 128], f8, isOutput=False)
    wphi_d = nc.declare_dram_parameter("wphi", [128, P, K], f8, isOutput=False)
    wplo_d = nc.declare_dram_parameter("wplo", [65, P, K], f8, isOutput=False)
    ncd_d = nc.declare_dram_parameter("ncd", [128, NKT, 128], f8, isOutput=False)
    wnc_d = nc.declare_dram_parameter("wnc", [128, NKT, HID], f8, isOutput=False)
    wout_d = nc.declare_dram_parameter("wout", [128, NFT + NFT_NC, OUT], bf16, isOutput=False)
    bout_d = nc.declare_dram_parameter("bout", [1, OUT], bf16, isOutput=False)
    out_d = nc.declare_dram_parameter("out", [NB, OUT], f32, isOutput=True)

    with tile.TileContext(nc) as tc:
        with (
            tc.tile_pool(name="cst", bufs=1) as cst,
            tc.tile_pool(name="sqp", bufs=2) as sqp,
            tc.tile_pool(name="tnp", bufs=2) as tnp,
            tc.tile_pool(name="s8p", bufs=2) as s8p,
            tc.tile_pool(name="htp", bufs=2) as htp,
            tc.tile_pool(name="tp", bufs=2, space="PSUM") as tp,
            tc.tile_pool(name="stp", bufs=2, space="PSUM") as stp,
            tc.tile_pool(name="ncp", bufs=1, space="PSUM") as ncp,
            tc.tile_pool(name="outp", bufs=1, space="PSUM") as outp,
        ):
            ident = cst.tile([128, 128], bf16)
            make_identity(nc, ident[:])
            ones_t = cst.tile([1, 128], bf16)
            nc.gpsimd.memset(ones_t[:], 1.0)

            # ---- persistent input tiles; all DMAs issued up front ----
            pdhi = cst.tile([128, NGRP, 8, 128], f8)
            pdlo = cst.tile([65, NGRP, 8, 128], f8)
            wphi_sb = cst.tile([128, P, K], f8)
            wplo_sb = cst.tile([65, P, K], f8)
            ncd = cst.tile([128, NKT, 128], f8)
            wnc = cst.tile([128, NKT, HID], f8)
            wout_sb = cst.tile([128, NFT + NFT_NC, OUT], bf16)
            bout_sb = cst.tile([1, OUT], bf16)

            # Issue order == rough arrival order == consumption order.
            nc.sync.dma_start(wout_sb[:], wout_d[:])
            nc.sync.dma_start(bout_sb[:], bout_d[:])
            # pathway data+weights in 4 chunks of 6-7 groups
            gsplit = [0, 7, 13, 19, NGRP]
            for ci in range(4):
                a, b = gsplit[ci], gsplit[ci + 1]
                pa, pb = 8 * a, 8 * b
                nc.sync.dma_start(pdhi[:, a:b], pdhi_d[:, a:b])
                nc.sync.dma_start(pdlo[:, a:b], pdlo_d[:, a:b])
                nc.sync.dma_start(wphi_sb[:, pa:pb], wphi_d[:, pa:pb])
                nc.sync.dma_start(wplo_sb[:, pa:pb], wplo_d[:, pa:pb])
            # nc branch in 5 chunks interleaved behind pathway data
            ksplit = [0, 24, 48, 72, 96, NKT]
            for ci in range(5):
                a, b = ksplit[ci], ksplit[ci + 1]
                nc.sync.dma_start(ncd[:, a:b], ncd_d[:, a:b])
                nc.sync.dma_start(wnc[:, a:b], wnc_d[:, a:b])

            t_all = cst.tile([NB, P, K], bf16)
            ss_all = cst.tile([NB, P], f32)
            inv_all = cst.tile([NB, P], f32)
            nc_ps = ncp.tile([NB, HID], f32)
            out_ps = outp.tile([NB, OUT], f32)

            # nc-branch DoubleRow matmul emitter: 59 kt-pairs x 2 hid halves
            nc_sched = []
            for i in range(NKP):
                for h in range(2):
                    nc_sched.append((i, h))
            nc_emitted = 0

            def emit_nc(upto):
                nonlocal nc_emitted
                while nc_emitted < min(upto, len(nc_sched)):
                    i, h = nc_sched[nc_emitted]
                    nc.tensor.matmul(
                        nc_ps[:, 256 * h:256 * h + 256],
                        ncd[:, 2 * i:2 * i + 2, :],
                        wnc[:, 2 * i:2 * i + 2, 256 * h:256 * h + 256],
                        start=(i == 0), stop=(i == NKP - 1),
                        perf_mode=DR,
                    )
                    nc_emitted += 1

            NSLOT = 2 * NGRP
            # ---------- phase A: pathway matmuls, square, reduce, stash t ----
            for g in range(NGRP):
                t_ps = tp.tile([NB, 8, K], f32)
                for j in range(8):
                    p = 8 * g + j
                    nc.tensor.matmul(t_ps[:, j, :], pdhi[:, g, j, :],
                                     wphi_sb[:, p, :], start=True, stop=False)
                    nc.tensor.matmul(t_ps[:, j, :], pdlo[:, g, j, :],
                                     wplo_sb[:, p, :], start=False, stop=True)
                sq = sqp.tile([NB, 8, K], bf16)
                nc.scalar.activation(sq[:], t_ps[:], AF.Square)
                nc.vector.tensor_reduce(ss_all[:, 8 * g:8 * g + 8], sq[:],
                                        axis=mybir.AxisListType.X,
                                        op=mybir.AluOpType.add)
                if g % 2 == 0:
                    nc.vector.tensor_copy(t_all[:, 8 * g:8 * g + 8, :], t_ps[:])
                else:
                    nc.scalar.copy(t_all[:, 8 * g:8 * g + 8, :], t_ps[:])
                emit_nc((g + 1) * len(nc_sched) // NSLOT)

            # ---------- phase B prep: 1/norm, batched (2 ACT table swaps) ----
            nc.scalar.sqrt(inv_all[:], ss_all[:])
            nc.vector.reciprocal(inv_all[:], inv_all[:])

            # ---------- phase B: normalize, sigmoid, transpose, W_out -------
            for g in range(NGRP):
                tn = tnp.tile([NB, 8, K], bf16)
                nc.vector.tensor_mul(
                    tn[:], t_all[:, 8 * g:8 * g + 8, :],
                    inv_all[:, 8 * g:8 * g + 8].broadcast_to((NB, 8, K)))
                s8 = s8p.tile([NB, 8, K], bf16)
                nc.scalar.activation(s8[:], tn[:], AF.Sigmoid)
                st_ps = stp.tile([128, 4, NB], f32)
                for jj in range(4):
                    nc.tensor.matmul(st_ps[:, jj, :], s8[:, 2 * jj:2 * jj + 2, :],
                                     ident[:], start=True, stop=True)
                hT = htp.tile([128, 4, NB], bf16)
                nc.vector.tensor_copy(hT[:, 0:2, :], st_ps[:, 0:2, :])
                nc.scalar.copy(hT[:, 2:4, :], st_ps[:, 2:4, :])
                for jj in range(4):
                    nc.tensor.matmul(out_ps[:], hT[:, jj, :],
                                     wout_sb[:, 4 * g + jj, :],
                                     start=(g == 0 and jj == 0), stop=False)
                emit_nc((NGRP + g + 1) * len(nc_sched) // NSLOT)

            # ---------- tail: nc sigmoid (undo x64 weight scale), out -------
            s_nc = cst.tile([NB, HID], bf16)
            nc.scalar.activation(s_nc[:], nc_ps[:], AF.Sigmoid, scale=1.0 / 64.0)
            for i in range(NFT_NC):
                st_ps = stp.tile([128, 4, NB], f32)
                nc.tensor.matmul(st_ps[:, 0, :], s_nc[:, i * 128:(i + 1) * 128],
                                 ident[:], start=True, stop=True)
                hT = htp.tile([128, 4, NB], bf16)
                if i % 2 == 0:
                    nc.vector.tensor_copy(hT[:, 0, :], st_ps[:, 0, :])
                else:
                    nc.scalar.copy(hT[:, 0, :], st_ps[:, 0, :])
                nc.tensor.matmul(out_ps[:], hT[:, 0, :], wout_sb[:, NFT + i, :],
                                 start=False, stop=False)
            nc.tensor.matmul(out_ps[:], ones_t[:], bout_sb[:],
                             start=False, stop=True)

            out_sb = cst.tile([NB, OUT], f32)
            nc.vector.tensor_copy(out_sb[:], out_ps[:])
            nc.sync.dma_start(out_d[:], out_sb[:])

    nc.compile()
    return nc


def _prep(inputs):
    x = np.asarray(inputs["x"], np.float32)
    pathway_ids = np.asarray(inputs["pathway_ids"]).astype(np.int64)
    nc_ids = np.asarray(inputs["nc_ids"]).astype(np.int64)
    W_path = np.asarray(inputs["W_path"], np.float32)
    b_path = np.asarray(inputs["b_path"], np.float32)
    W_nc = np.asarray(inputs["W_nc"], np.float32)
    b_nc = np.asarray(inputs["b_nc"], np.float32)
    W_out = np.asarray(inputs["W_out"], np.float32)
    b_out = np.asarray(inputs["b_out"], np.float32)

    n = x.shape[0]
    xt = np.ascontiguousarray(x.reshape(n, G * C).T)            # [60000, n]
    xf = xt.astype(F8)

    # pathway gather: contraction row i of pathway p = gene pathway_ids[p, i//3], channel i%3
    pidx = ((pathway_ids * 3)[:, :, None] + np.arange(3)).reshape(P, KC)
    prows = xf[pidx.reshape(-1)].reshape(P, KC, n)              # [200, 192, n]
    ph = prows[:, 0:128, :]                                     # [200, 128, n]
    pl = np.concatenate([prows[:, 128:KC, :],
                         np.ones((P, 1, n), F8)], axis=1)       # [200, 65, n]

    # weights x8 (cancelled exactly by the per-pathway L2 normalize)
    w8 = (8.0 * W_path).astype(np.float32)                      # [200, 192, 64]
    wphi = np.ascontiguousarray(w8[:, 0:128, :].transpose(1, 0, 2)).astype(F8)
    wplo = np.ascontiguousarray(
        np.concatenate([w8[:, 128:KC, :], (8.0 * b_path)[:, None, :]], axis=1)
        .transpose(1, 0, 2)).astype(F8)                         # [65, 200, 64]

    # nc gather rows (+ ones row for bias, zero-pad to NKROWS)
    nidx = ((nc_ids * 3)[:, None] + np.arange(3)).reshape(-1)
    ncr = np.zeros((NKROWS, n), F8)
    ncr[:NCG * C] = xf[nidx]
    ncr[NCG * C] = 1.0

    wnca = np.zeros((NKROWS, HID), np.float32)
    wnca[:NCG * C] = 64.0 * W_nc
    wnca[NCG * C] = 64.0 * b_nc
    wnc8 = np.ascontiguousarray(
        wnca.reshape(NKT, 128, HID).transpose(1, 0, 2)).astype(F8)  # [128, 118, 512]

    wout_t = np.ascontiguousarray(
        W_out.reshape(NFT + NFT_NC, 128, OUT).transpose(1, 0, 2)).astype(BF)
    bout = b_out.reshape(1, OUT).astype(BF)

    in_maps = []
    for c in range(NCORES):
        sl = slice(c * NB, (c + 1) * NB)
        pdhi = np.ascontiguousarray(
            ph[:, :, sl].transpose(1, 0, 2)).reshape(128, NGRP, 8, 128)
        pdlo = np.ascontiguousarray(
            pl[:, :, sl].transpose(1, 0, 2)).reshape(65, NGRP, 8, 128)
        ncd = np.ascontiguousarray(
            ncr[:, sl].reshape(NKT, 128, NB).transpose(1, 0, 2))  # [128, 118, 128]
        in_maps.append({
            "pdhi": pdhi,
            "pdlo": pdlo,
            "wphi": wphi,
            "wplo": wplo,
            "ncd": ncd,
            "wnc": wnc8,
            "wout": wout_t,
            "bout": bout,
        })
    return in_maps


def kernel(**inputs):
    if "nc" not in _CACHE:
        _CACHE["nc"] = _build()
    nc = _CACHE["nc"]
    in_maps = _prep(inputs)
    res = run_bass_kernel_spmd(nc, in_maps, list(range(NCORES)), **_CACHE.get("run_kwargs", {}))
    _CACHE["last_result"] = res
    return np.concatenate([res.results[c]["out"] for c in range(NCORES)], axis=0)


if __name__ == "__main__":
    print("building only...")
    _build()
    print("build OK")
